# revision 2
# baseline (speedup 1.0000x reference)
"""DRRN scoring network, v4: per-encoder sharding on 8 NeuronCores.

Two-dispatch design. Phase 1 (encoders): each core runs ONE GRU encoder
over 128 rows x 256 steps -- cores 0-1 obs, 2-3 look, 4-5 inv, 6-7 act
(act runs 8 sequential 32-step chunks of 128 rows). Full-width matmuls
(M=128) replace the baseline's 32-col tiling: 8 matmuls + 2 dma-xbar
transposes per step instead of 27 matmuls + 2 PE transposes. Each core
also computes its encoder's MLP contribution c_e = h @ W_e^T per chunk
slot. Phase 2 (MLP): data-parallel over states; host only slices /
replicates phase-1 outputs between dispatches (no host flops).
"""
import numpy as np
import ml_dtypes
import concourse.bacc as bacc
import concourse.mybir as mybir
from concourse.tile import TileContext
from concourse.bass_utils import run_bass_kernel_spmd

dt = mybir.dt
F32, BF16, I16, F32R = dt.float32, dt.bfloat16, dt.int16, dt.float32r
bf = ml_dtypes.bfloat16

V, E, H = 32000, 128, 256
B, S = 256, 256
A, SA = 8, 32
NCORES = 8
NSTEP = S
NIDX = NSTEP * 128
NSLAB = 8
SLAB = NIDX // NSLAB
H3, H2 = 3 * H, 2 * H

Sig = mybir.ActivationFunctionType.Sigmoid
Tanh = mybir.ActivationFunctionType.Tanh
Relu = mybir.ActivationFunctionType.Relu
Ident = mybir.ActivationFunctionType.Identity
MUL = mybir.AluOpType.mult
ADD = mybir.AluOpType.add
SUB = mybir.AluOpType.subtract


def build_enc(nreps=1):
    nc = bacc.Bacc("TRN2", target_bir_lowering=False, debug=False)

    d_emb = nc.declare_dram_parameter("embb", [V, E], BF16, isOutput=False)
    d_idx = nc.declare_dram_parameter("idx", [128, NIDX // 16], I16, isOutput=False)
    d_wih = nc.declare_dram_parameter("wihT", [E, H3], BF16, isOutput=False)
    d_whh = nc.declare_dram_parameter("whhT", [128, 2, H3], BF16, isOutput=False)
    d_sel = nc.declare_dram_parameter("sel", [4, 128], F32R, isOutput=False)
    d_brz = nc.declare_dram_parameter("brz", [4, H2], F32R, isOutput=False)
    d_bnn = nc.declare_dram_parameter("bnn", [4, H2], F32R, isOutput=False)
    d_m = nc.declare_dram_parameter("mask", [128, NSTEP], F32, isOutput=False)
    d_rm = nc.declare_dram_parameter("rmask", [128, 1], F32, isOutput=False)
    d_rbt = nc.declare_dram_parameter("rbt", [128, H], BF16, isOutput=False)
    d_hw = nc.declare_dram_parameter("hWT", [128, 2, H], BF16, isOutput=False)
    d_c = nc.declare_dram_parameter("contrib", [A, 128, H], F32, isOutput=True)

    with TileContext(nc) as tc:
        with tc.tile_pool(name="w", bufs=1) as wp, \
             tc.tile_pool(name="xp", bufs=1) as xp, \
             tc.tile_pool(name="st", bufs=1) as stp, \
             tc.tile_pool(name="rot", bufs=3) as rp, \
             tc.tile_pool(name="ps", bufs=2, space="PSUM") as ps:

            t_idx = wp.tile([128, NIDX // 16], I16, name="t_idx")
            nc.sync.dma_start(out=t_idx[:], in_=d_idx[:])
            t_wih = wp.tile([E, H3], BF16, name="t_wih")
            nc.sync.dma_start(out=t_wih[:], in_=d_wih[:])
            t_whh = wp.tile([128, 2, H3], BF16, name="t_whh")
            nc.sync.dma_start(out=t_whh[:], in_=d_whh[:])
            t_sel = wp.tile([4, 128], F32R, name="t_sel")
            nc.sync.dma_start(out=t_sel[:], in_=d_sel[:])
            t_brz = wp.tile([4, H2], F32R, name="t_brz")
            nc.sync.dma_start(out=t_brz[:], in_=d_brz[:])
            t_bnn = wp.tile([4, H2], F32R, name="t_bnn")
            nc.sync.dma_start(out=t_bnn[:], in_=d_bnn[:])
            t_m = wp.tile([128, NSTEP], F32, name="t_m")
            nc.sync.dma_start(out=t_m[:], in_=d_m[:])
            t_rm = wp.tile([128, 1], F32, name="t_rm")
            nc.sync.dma_start(out=t_rm[:], in_=d_rm[:])
            t_rbt = wp.tile([128, H], BF16, name="t_rbt")
            nc.sync.dma_start(out=t_rbt[:], in_=d_rbt[:])
            t_hw = wp.tile([128, 2, H], BF16, name="t_hw")
            nc.sync.dma_start(out=t_hw[:], in_=d_hw[:])

            slots = [stp.tile([128, H], BF16, tag=f"slot{g}", name=f"slot{g}")
                     for g in range(A)]

            for rep in range(nreps):
                xts = []
                for s in range(NSLAB):
                    xt = xp.tile([128, 1, SLAB], BF16, tag=f"xt{s}", name=f"xt{s}")
                    nc.gpsimd.dma_gather(
                        out_ap=xt[:], in_ap=d_emb[:],
                        idxs_ap=t_idx[:, (SLAB // 16) * s:(SLAB // 16) * (s + 1)],
                        num_idxs=SLAB, num_idxs_reg=SLAB, elem_size=E,
                        transpose=True, single_packet=False,
                    )
                    xts.append(xt)
                h_A = rp.tile([128, H], BF16, tag="hA", name="hA")
                nc.vector.memset(h_A[:], 0.0)
                h_T = rp.tile([128, H], BF16, tag="hT", name="hT")
                nc.vector.memset(h_T[:], 0.0)

                for t in range(NSTEP):
                    xT = xts[t // SA][:, 0, (t % SA) * 128:(t % SA + 1) * 128]
                    p1 = ps.tile([128, H2], F32, tag="p1", name="p1")
                    p23 = ps.tile([128, H2], F32, tag="p23", name="p23")
                    nc.tensor.matmul(p1[:], t_sel[:], t_brz[:], start=True, stop=False)
                    nc.tensor.matmul(p23[:], t_sel[:], t_bnn[:], start=True, stop=False)
                    nc.tensor.matmul(p1[:], xT, t_wih[:, 0:H2], start=False, stop=False)
                    nc.tensor.matmul(p23[:, 0:H], xT, t_wih[:, H2:H3],
                                     start=False, stop=True)
                    nc.tensor.matmul(p1[:], h_T[:, 0:128], t_whh[:, 0, 0:H2],
                                     start=False, stop=False)
                    nc.tensor.matmul(p23[:, H:H2], h_T[:, 0:128], t_whh[:, 0, H2:H3],
                                     start=False, stop=False, skip_group_check=True)
                    nc.tensor.matmul(p1[:], h_T[:, 128:256], t_whh[:, 1, 0:H2],
                                     start=False, stop=True)
                    nc.tensor.matmul(p23[:, H:H2], h_T[:, 128:256], t_whh[:, 1, H2:H3],
                                     start=False, stop=True, skip_group_check=True)

                    s_r = rp.tile([128, H], BF16, tag="sr", name="s_r")
                    nc.scalar.activation(s_r[:], p1[:, 0:H], Sig)
                    s_zb = rp.tile([128, H], BF16, tag="szb", name="s_zb")
                    nc.scalar.activation(s_zb[:], p1[:, H:H2], Sig, scale=-1.0)
                    t1 = rp.tile([128, H], BF16, tag="t1", name="t1")
                    nc.vector.tensor_tensor(t1[:], s_r[:], p23[:, H:H2], MUL)
                    t2 = rp.tile([128, H], BF16, tag="t2", name="t2")
                    nc.vector.tensor_tensor(t2[:], t1[:], p23[:, 0:H], ADD)
                    s_n = rp.tile([128, H], BF16, tag="sn", name="s_n")
                    nc.scalar.activation(s_n[:], t2[:], Tanh)
                    s_d = rp.tile([128, H], BF16, tag="sd", name="s_d")
                    nc.vector.tensor_tensor(s_d[:], s_n[:], h_A[:], SUB)
                    s_u = rp.tile([128, H], BF16, tag="su", name="s_u")
                    nc.vector.scalar_tensor_tensor(s_u[:], s_zb[:], t_m[:, t:t + 1],
                                                   s_d[:], MUL, MUL)
                    h_A2 = rp.tile([128, H], BF16, tag="hA", name="hA")
                    nc.vector.tensor_tensor(h_A2[:], h_A[:], s_u[:], ADD)

                    if t % SA == SA - 1:
                        g = t // SA
                        hTp = rp.tile([128, H], BF16, tag="hTp", name="hTp")
                        for k in range(2):
                            nc.sync.dma_start_transpose(
                                out=hTp[:, 128 * k:128 * (k + 1)],
                                in_=h_A2[:, 128 * k:128 * (k + 1)])
                        nc.vector.tensor_copy(slots[g][:], hTp[:])
                        if t != NSTEP - 1:
                            h_T2 = rp.tile([128, H], BF16, tag="hT", name="hT")
                            nc.vector.tensor_tensor(h_T2[:], hTp[:], t_rbt[:], MUL)
                            h_T = h_T2
                            h_A3 = rp.tile([128, H], BF16, tag="hA", name="hA")
                            nc.vector.tensor_scalar_mul(h_A3[:], h_A2[:], t_rm[:, 0:1])
                            h_A = h_A3
                    else:
                        h_T2 = rp.tile([128, H], BF16, tag="hT", name="hT")
                        for k in range(2):
                            nc.sync.dma_start_transpose(
                                out=h_T2[:, 128 * k:128 * (k + 1)],
                                in_=h_A2[:, 128 * k:128 * (k + 1)])
                        h_T = h_T2
                        h_A = h_A2

                for g in range(A):
                    pc = ps.tile([128, H], F32, tag="pc", name="pc")
                    nc.tensor.matmul(pc[:], slots[g][:, 0:128], t_hw[:, 0, :],
                                     start=True, stop=False)
                    nc.tensor.matmul(pc[:], slots[g][:, 128:256], t_hw[:, 1, :],
                                     start=False, stop=True)
                    c_s = rp.tile([128, H], F32, tag="cs", name="c_s")
                    nc.scalar.activation(c_s[:], pc[:], Ident)
                    nc.sync.dma_start(out=d_c[g], in_=c_s[:])

    nc.compile()
    return nc


def build_mlp(nreps=1):
    nc = bacc.Bacc("TRN2", target_bir_lowering=False, debug=False)

    d_obs = nc.declare_dram_parameter("cobs", [128, 2, H], BF16, isOutput=False)
    d_look = nc.declare_dram_parameter("clook", [128, 2, H], BF16, isOutput=False)
    d_inv = nc.declare_dram_parameter("cinv", [128, 2, H], BF16, isOutput=False)
    d_act = nc.declare_dram_parameter("cact", [128, 2, H], BF16, isOutput=False)
    d_hbt = nc.declare_dram_parameter("hbt", [128, H], BF16, isOutput=False)
    d_scb = nc.declare_dram_parameter("scorerb", [128, H], BF16, isOutput=False)
    d_sbt = nc.declare_dram_parameter("sbt", [128, 1], F32, isOutput=False)
    d_q = nc.declare_dram_parameter("q", [2, 128], F32, isOutput=True)

    with TileContext(nc) as tc:
        with tc.tile_pool(name="w", bufs=1) as wp, \
             tc.tile_pool(name="rot", bufs=2) as rp:
            t_obs = wp.tile([128, 2, H], BF16, name="t_obs")
            nc.sync.dma_start(out=t_obs[:], in_=d_obs[:])
            t_look = wp.tile([128, 2, H], BF16, name="t_look")
            nc.sync.dma_start(out=t_look[:], in_=d_look[:])
            t_inv = wp.tile([128, 2, H], BF16, name="t_inv")
            nc.sync.dma_start(out=t_inv[:], in_=d_inv[:])
            t_act = wp.tile([128, 2, H], BF16, name="t_act")
            nc.sync.dma_start(out=t_act[:], in_=d_act[:])
            t_hbt = wp.tile([128, H], BF16, name="t_hbt")
            nc.sync.dma_start(out=t_hbt[:], in_=d_hbt[:])
            t_scb = wp.tile([128, H], BF16, name="t_scb")
            nc.sync.dma_start(out=t_scb[:], in_=d_scb[:])
            t_sbt = wp.tile([128, 1], F32, name="t_sbt")
            nc.sync.dma_start(out=t_sbt[:], in_=d_sbt[:])

            for rep in range(nreps):
                for k in range(2):
                    a1 = rp.tile([128, H], BF16, tag="a1", name="a1")
                    nc.vector.tensor_tensor(a1[:], t_obs[:, k, :], t_look[:, k, :], ADD)
                    a2 = rp.tile([128, H], BF16, tag="a2", name="a2")
                    nc.vector.tensor_tensor(a2[:], t_inv[:, k, :], t_act[:, k, :], ADD)
                    a3 = rp.tile([128, H], BF16, tag="a3", name="a3")
                    nc.vector.tensor_tensor(a3[:], a1[:], a2[:], ADD)
                    zp = rp.tile([128, H], BF16, tag="zp", name="zp")
                    nc.vector.tensor_tensor(zp[:], a3[:], t_hbt[:], ADD)
                    z = rp.tile([128, H], BF16, tag="z", name="z")
                    nc.scalar.activation(z[:], zp[:], Relu)
                    qm = rp.tile([128, H], F32, tag="qm", name="qm")
                    nc.vector.tensor_tensor(qm[:], z[:], t_scb[:], MUL)
                    qv = rp.tile([128, 1], F32, tag="qv", name="qv")
                    nc.vector.reduce_sum(qv[:], qm[:], axis=mybir.AxisListType.X)
                    qf = rp.tile([128, 1], F32, tag="qf", name="qf")
                    nc.vector.tensor_scalar_add(qf[:], qv[:], t_sbt[:, 0:1])
                    nc.sync.dma_start(out=d_q[k], in_=qf[:, 0])

    nc.compile()
    return nc


def _wrap_idx(tokens_flat):
    out = np.zeros((128, NIDX // 16), np.int16)
    for s in range(NSLAB):
        blk = tokens_flat[SLAB * s:SLAB * (s + 1)].reshape(SLAB // 16, 16).T
        out[:, (SLAB // 16) * s:(SLAB // 16) * (s + 1)] = np.tile(blk, (8, 1))
    return out


def prep_enc(obs_tokens, obs_len, look_tokens, look_len, inv_tokens, inv_len,
             act_tokens, act_len, emb, Wih, Whh, bih, bhh,
             hidden_W, hidden_b, scorer_W, scorer_b):
    npf = np.asarray
    enc_tok = [npf(obs_tokens), npf(look_tokens), npf(inv_tokens)]
    enc_len = [np.maximum(npf(obs_len), 1), np.maximum(npf(look_len), 1),
               np.maximum(npf(inv_len), 1)]
    act_tokens = npf(act_tokens)
    act_len = np.maximum(npf(act_len), 1)
    emb = npf(emb, np.float32)
    Wih = npf(Wih, np.float32)
    Whh = npf(Whh, np.float32)
    bih = npf(bih, np.float32)
    bhh = npf(bhh, np.float32)
    hidden_W = npf(hidden_W, np.float32)

    emb_bf = emb.astype(bf)
    sel = np.zeros((4, 128), np.float32)
    sel[0, :] = 1.0

    in_maps = []
    for c in range(NCORES):
        e = c // 2
        hf = c % 2
        wihT = np.ascontiguousarray(Wih[e].T).astype(bf)            # [E, 768]
        whhT = np.ascontiguousarray(
            Whh[e].T.reshape(2, 128, H3).transpose(1, 0, 2)).astype(bf)
        brz = np.zeros((4, H2), np.float32)
        brz[0] = bih[e, 0:H2] + bhh[e, 0:H2]
        bnn = np.zeros((4, H2), np.float32)
        bnn[0, 0:H] = bih[e, H2:H3]
        bnn[0, H:H2] = bhh[e, H2:H3]
        hWT = np.ascontiguousarray(
            hidden_W[:, H * e:H * (e + 1)].T.reshape(2, 128, H)
            .transpose(1, 0, 2)).astype(bf)                         # [128,2,H]

        toks = np.zeros((NSTEP, 128), np.int64)
        m = np.zeros((128, NSTEP), np.float32)
        if e < 3:
            seqs = enc_tok[e][128 * hf:128 * (hf + 1)]               # [128, S]
            lens = enc_len[e][128 * hf:128 * (hf + 1)]
            toks[:, :] = seqs.T
            m[:, :] = (np.arange(NSTEP)[None, :] < lens[:, None])
            rmv = 1.0
        else:
            at = act_tokens[1024 * hf:1024 * (hf + 1)]               # [1024, SA]
            al = act_len[1024 * hf:1024 * (hf + 1)]
            for g in range(A):
                toks[SA * g:SA * (g + 1), :] = at[128 * g:128 * (g + 1)].T
                m[:, SA * g:SA * (g + 1)] = (
                    np.arange(SA)[None, :] < al[128 * g:128 * (g + 1)][:, None])
            rmv = 0.0
        in_maps.append({
            "embb": emb_bf,
            "idx": _wrap_idx(toks.reshape(-1)),
            "wihT": wihT, "whhT": whhT, "sel": sel,
            "brz": brz, "bnn": bnn, "mask": m,
            "rmask": np.full((128, 1), rmv, np.float32),
            "rbt": np.full((128, H), rmv, np.float32).astype(bf),
            "hWT": hWT,
        })
    return in_maps


def prep_mlp(contribs, hidden_b, scorer_W, scorer_b):
    """contribs: list of 8 arrays [A, 128, H] f32 (per encoder core).
    Pure slicing/replication -- no arithmetic."""
    hidden_b = np.asarray(hidden_b, np.float32)
    scorer_W = np.asarray(scorer_W, np.float32)
    scorer_b = np.asarray(scorer_b, np.float32)
    hbt = np.tile(hidden_b.reshape(1, H), (128, 1)).astype(bf)
    scb = np.tile(scorer_W.reshape(1, H), (128, 1)).astype(bf)
    sbt = np.full((128, 1), float(scorer_b.reshape(-1)[0]), np.float32)

    in_maps = []
    for c in range(NCORES):
        quarter = c // 4          # which half-core (0: cores 0/2/4, 1: cores 1/3/5)
        row0 = 32 * (c % 4)       # state rows within that core's slot-7 block
        srows = (row0 + np.arange(256) // A)                 # per (s,a) row
        cs = {}
        for name, enc_core in (("cobs", 0), ("clook", 2), ("cinv", 4)):
            src = contribs[enc_core + quarter][A - 1]        # slot 7 [128, H]
            rep = src[srows]                                 # [256, H]
            cs[name] = np.ascontiguousarray(rep.reshape(2, 128, H).transpose(1, 0, 2)).astype(bf)
        acore = 6 + quarter
        g0 = 2 * (c % 4)
        cact = np.concatenate([contribs[acore][g0], contribs[acore][g0 + 1]],
                              axis=0)                        # [256, H]
        in_maps.append({
            **cs,
            "cact": np.ascontiguousarray(cact.reshape(2, 128, H).transpose(1, 0, 2)).astype(bf),
            "hbt": hbt, "scorerb": scb, "sbt": sbt,
        })
    return in_maps


_NC_CACHE = {}


def kernel(**inputs):
    if "enc" not in _NC_CACHE:
        _NC_CACHE["enc"] = build_enc(1)
        _NC_CACHE["mlp"] = build_mlp(1)
    nc_e, nc_m = _NC_CACHE["enc"], _NC_CACHE["mlp"]

    enc_maps = prep_enc(**inputs)
    res_e = run_bass_kernel_spmd(nc_e, enc_maps, list(range(NCORES)))
    contribs = [np.asarray(res_e.results[c]["contrib"], np.float32)
                for c in range(NCORES)]

    mlp_maps = prep_mlp(contribs, inputs["hidden_b"], inputs["scorer_W"],
                        inputs["scorer_b"])
    res_m = run_bass_kernel_spmd(nc_m, mlp_maps, list(range(NCORES)))
    q = np.concatenate([np.asarray(res_m.results[c]["q"], np.float32).reshape(-1)
                        for c in range(NCORES)])
    return q.reshape(B, A)
